# revision 57
# baseline (speedup 1.0000x reference)
"""Trainium2 Bass kernel for a post-LN transformer encoder layer.

Reference computation (fp32, per batch b):
    q,k,v = x@Wq+bq, x@Wk+bk, x@Wv+bv          (D=1024, H=16 heads, dk=64)
    attn  = softmax(q k^T / sqrt(dk)) v         (S=2048, mask is all-ones)
    h     = LN(x + attn@Wo + bo; g1, be1)
    out   = LN(h + relu(h@W1+b1)@W2 + b2; g2, be2)

Sharding: 8 cores, fully independent (no collectives). Core c owns batch
b=c//2, sequence half c%2 (1024 query tokens), and redundantly computes
K/V for its full batch (2048 keys) from a host-provided transposed copy
of x. The host rolls the token axis so each core's local tokens come
first (attention is permutation-invariant over keys).

fp8 scheme: all big matmuls run fp8e4m3 in DoubleRow perf mode (0.5
cycles/row = 2x PE throughput; scores stay normal-mode fp8 since their
contraction dim dk=64 lives on partitions). Operands carry power-of-2
scales chosen so every eviction needs at most a bias add (no ACT scale):
s_x*s_w == s_q and s_w1*s_h == s_u, letting Q/K land via ACT
Identity+bias, V via DVE copy, and relu via DVE (ps+bias max 0). The
layernorm outputs absorb the residual scales (LN is invariant to input
scaling once eps is scaled to match). Exp tiles are raw exp values in
fp8 (score sigma is ~0.33 after the 1/sqrt(dk) fold, so exp in
[~0.25, ~4] fits e4m3); the softmax denominator rides as an all-ones V
column so et/V scales cancel exactly in ctx/denom.

Scales (log2): x:2 W(qkvo):4 q/k/v/ctx:6 hres:10 h(postLN1):15 hT:1
W1:5 u:6 W2:9; ffn psum 6+9=15 matches h; eps1=1e-5*2^20 eps2=1e-5*2^30.

Schedule (emission order = per-engine queue order; Tile adds the deps):
  p1: Q,K (DoubleRow, ACT evict, two 4-psum output groups) then V
      (DoubleRow, DVE evict + ones col); w2 half A prefetch.
  attention: paired-head units (hc,qt,half), qt-major, software-pipelined
    scores (fp8, K=64) -> exp (ACT, fp8 out) -> ctx (DoubleRow kc-pairs)
    -> recip tail (DVE recip + PE ones-bcast + DVE mul -> CT fp8).
    Filler closures are injected into late units so PE/DVE gaps under the
    ACT-bound exp stream do useful work: p3 for qt0 tokens (out-proj
    DoubleRow + residual + LN1 + PE-transpose -> hT), FFN1 qt0
    (DoubleRow + DVE relu -> uT), FFN2 + LN2 + output DMA for qt0 tokens.
  tail: p3 qt1 -> hT, FFN1 qt1, FFN2 + LN2 for qt1 tokens.
ACT's in-order queue stays essentially pure exp; LN rsqrt ops are emitted
right after a unit's exp burst so their DVE inputs are ready when reached.
"""

import numpy as np
import ml_dtypes

import concourse.bass as bass
import concourse.mybir as mybir
import concourse.tile as tile
from concourse.bass import ts
from concourse.bass_utils import run_bass_kernel_spmd
from concourse.masks import make_identity

BF16 = mybir.dt.bfloat16
F32 = mybir.dt.float32
F32R = mybir.dt.float32r
FP8 = mybir.dt.float8e4
AF = mybir.ActivationFunctionType
ALU = mybir.AluOpType
DR = mybir.MatmulPerfMode.DoubleRow

D = 1024
DFF = 4096
H = 16
DK = 64
S_FULL = 2048
S_LOC = 1024
P = 128
NDC = D // P        # 8  feature chunks
NFC = DFF // P      # 32 ffn chunks
NKC = S_FULL // P   # 16 key chunks
NTC = S_LOC // P    # 8  local token chunks
NQT = S_LOC // 512  # 2 query tiles of 512
NKT = S_FULL // 512 # 4 key-token tiles of 512

# scales (log2); SX+SW == SQ(==SV) and SW1+SH == SU so evictions skip scales
SX = 2
SW = 4
SQ = 6
SV = 6
SW1 = 5
SH = 1
SU = 6
SW2 = 9
SRES = SV + SW   # 10: hres scale pre-LN1 (ctx@Wo psum scale)
SLN = SU + SW2   # 15: post-LN1 h scale == FFN2 psum scale
EPS1 = 1e-5 * 2.0 ** (2 * SRES)
EPS2 = 1e-5 * 2.0 ** (2 * SLN)


# ---------------------------------------------------------------------------
# Multi-wait splitting: this walrus build rejects instructions carrying more
# than one sync-wait command. Tile occasionally emits several (notably the
# kernel-tail drain). Keep the last wait on the instruction and hoist the
# rest onto NoOps inserted just before it on the same engine queue.
_ctr = [0]


def _split_block(bb):
    out = []
    changed = False
    for inst in bb.instructions:
        si = inst.sync_info
        waits = list(si.on_wait) if si is not None and si.on_wait else []
        if len(waits) > 1:
            changed = True
            for w in waits[:-1]:
                _ctr[0] += 1
                nop = mybir.InstNoOp(name=f"waitfix-{_ctr[0]}", ins=[], outs=[])
                nop.engine = inst.engine
                nop.sync_info = mybir.SyncInfo(on_wait=[w], on_update=[])
                out.append(nop)
            inst.sync_info = mybir.SyncInfo(
                on_wait=[waits[-1]], on_update=list(si.on_update or [])
            )
        out.append(inst)
    if changed:
        bb.instructions = out
    return changed


def fix_multiwait(nc):
    for fn in nc.m.functions:
        for bb in fn.blocks:
            _split_block(bb)


# ---------------------------------------------------------------------------
def build_program(reps=1, waitfix=True, ln_trivial=False):
    # the deferred Q/K fillers need extra vp buffers that only fit in the
    # ln_trivial layout (the gamma/beta broadcast tiles are dropped there)
    DEFER_K = ln_trivial
    nc = bass.Bass()

    xt_d = nc.dram_tensor("xt", [D, S_FULL], FP8, kind="ExternalInput")
    xloc_d = nc.dram_tensor("xloc", [S_LOC, D], F32, kind="ExternalInput")
    wq_d = nc.dram_tensor("wq", [D, D], FP8, kind="ExternalInput")
    wk_d = nc.dram_tensor("wk", [D, D], FP8, kind="ExternalInput")
    wv_d = nc.dram_tensor("wv", [D, D], FP8, kind="ExternalInput")
    wo_d = nc.dram_tensor("wo", [D, D], FP8, kind="ExternalInput")
    w1_d = nc.dram_tensor("w1", [D, DFF], FP8, kind="ExternalInput")
    w2_d = nc.dram_tensor("w2", [DFF, D], FP8, kind="ExternalInput")
    bqc_d = nc.dram_tensor("bqc", [P, NDC], F32, kind="ExternalInput")
    bkc_d = nc.dram_tensor("bkc", [P, NDC], F32, kind="ExternalInput")
    b1c_d = nc.dram_tensor("b1c", [P, NFC], F32, kind="ExternalInput")
    b2r_d = nc.dram_tensor("b2r", [1, D], F32, kind="ExternalInput")
    g1r_d = nc.dram_tensor("g1r", [1, D], F32, kind="ExternalInput")
    be1r_d = nc.dram_tensor("be1r", [1, D], F32, kind="ExternalInput")
    g2r_d = nc.dram_tensor("g2r", [1, D], F32, kind="ExternalInput")
    be2r_d = nc.dram_tensor("be2r", [1, D], F32, kind="ExternalInput")
    out_d = nc.dram_tensor("out", [S_LOC, D], F32, kind="ExternalOutput")

    xt_r = xt_d.rearrange("(dc p) t -> p dc t", p=P)
    wq_r = wq_d.rearrange("(dc p) o -> p dc o", p=P)
    wk_r = wk_d.rearrange("(dc p) o -> p dc o", p=P)
    wv_r = wv_d.rearrange("(dc p) o -> p dc o", p=P)
    wo_r = wo_d.rearrange("(dc p) o -> p dc o", p=P)
    w1_r = w1_d.rearrange("(dc p) f -> p dc f", p=P)
    w2_r = w2_d.rearrange("(fc p) o -> p fc o", p=P)

    def bcast_row(row_d):
        # [1, D] dram row -> partition-broadcast AP for DMA into [P, D]
        a = row_d[0:1, :]
        return bass.AP(tensor=a.tensor, offset=a.offset, ap=[[0, P], [1, D]])

    def layernorm_row(row, lnp, g_b, be_b, eps_t, sqrt_scale=1.0):
        # With ln_trivial (gamma==1, beta==0 detected host-side), the output
        # rescale folds into the Sqrt activation scale: r = recip(sqrt(
        # sqrt_scale*var + bias)) makes (row-mu)*r directly the desired
        # output, and the gamma/beta tensor ops are skipped entirely.
        st = lnp.tile([P, 2, 6], F32, tag="st")
        nc.vector.bn_stats(st[:, 0, :], row[:, 0:512])
        nc.vector.bn_stats(st[:, 1, :], row[:, 512:1024])
        mv = lnp.tile([P, 2], F32, tag="mv")
        nc.vector.bn_aggr(mv[:], st[:])
        nc.scalar.activation(mv[:, 1:2], mv[:, 1:2], AF.Sqrt, bias=eps_t[:],
                             scale=sqrt_scale)
        nc.vector.reciprocal(mv[:, 1:2], mv[:, 1:2])
        nc.vector.tensor_scalar(
            out=row,
            in0=row,
            scalar1=mv[:, 0:1],
            scalar2=mv[:, 1:2],
            op0=ALU.subtract,
            op1=ALU.mult,
        )
        if not ln_trivial:
            nc.vector.tensor_mul(row, row, g_b[:])
            nc.vector.tensor_add(row, row, be_b[:])

    with tile.TileContext(nc) as tc:
        with (
            tc.tile_pool(name="top", bufs=1) as top,
            tc.tile_pool(name="lnp", bufs=2) as lnp,
        ):
            # ---- whole-kernel constants -----------------------------------
            ident = top.tile([P, P], F32)
            make_identity(nc, ident)
            eps1_t = top.tile([P, 1], F32)
            eps2_t = top.tile([P, 1], F32)
            nc.vector.memset(eps2_t, EPS2)
            if ln_trivial:
                # LN1 sqrt computes 2^-SLN+SRES... r folds the 2^SLN output
                # scale: sqrt(2^(-2SLN)*var_s + 1e-5*2^(2SRES-2SLN))
                nc.vector.memset(eps1_t, 1e-5 * 2.0 ** (2 * SRES - 2 * SLN))
                ln1_sqrt_scale = 2.0 ** (-2 * SLN)
            else:
                nc.vector.memset(eps1_t, EPS1)
                ln1_sqrt_scale = 1.0
            ones32 = top.tile([1, DK], F32)
            nc.vector.memset(ones32, 1.0)
            ones_r = top.tile([1, DK], F32R)
            with nc.allow_low_precision(reason="f32r round for PE broadcast"):
                nc.vector.tensor_copy(ones_r[:], ones32[:])
            bqc = top.tile([P, NDC], F32)
            nc.sync.dma_start(bqc[:], bqc_d[:])
            bkc = top.tile([P, NDC], F32)
            nc.sync.dma_start(bkc[:], bkc_d[:])
            b1c = top.tile([P, NFC], F32)
            if ln_trivial:
                b2b = g1b = be1b = g2b = be2b = None

                def late_const_dmas():
                    nc.gpsimd.dma_start(b1c[:], b1c_d[:])
            else:
                b2b = top.tile([P, D], F32)
                g1b = top.tile([P, D], F32)
                be1b = top.tile([P, D], F32)
                g2b = top.tile([P, D], F32)
                be2b = top.tile([P, D], F32)

                def late_const_dmas():
                    nc.gpsimd.dma_start(b1c[:], b1c_d[:])
                    nc.gpsimd.dma_start(b2b[:], bcast_row(b2r_d))
                    nc.gpsimd.dma_start(g1b[:], bcast_row(g1r_d))
                    nc.gpsimd.dma_start(be1b[:], bcast_row(be1r_d))
                    nc.gpsimd.dma_start(g2b[:], bcast_row(g2r_d))
                    nc.gpsimd.dma_start(be2b[:], bcast_row(be2r_d))

            for _rep in range(reps):
              with (
                tc.tile_pool(name="repp", bufs=1) as repp,
                tc.tile_pool(name="w1p", bufs=2) as w1p,
                tc.tile_pool(name="wop", bufs=1) as wop,
              ):
                # persistents that cross the attention/tail boundary
                CTs = [repp.tile([P, NDC, 512], FP8, name=f"CT{i}") for i in range(NQT)]
                hress = [repp.tile([P, 4, D], F32, name=f"hres{i}") for i in range(NQT)]
                hTs = [repp.tile([P, NDC, 512], FP8, name=f"hT{i}") for i in range(NQT)]
                uT0 = repp.tile([P, NFC, 512], FP8)
                w2_sbs = [repp.tile([P, NFC, 512], FP8, name=f"w2h{i}") for i in range(2)]
                wo_sb = wop.tile([P, NDC, D], FP8)

                # ---- chunk emitters (fillers during attention + tail) -----
                # psa(shape, name) allocates a psum tile from the active ring
                def p3_proj(qt, tci, psa_o):
                    # out-proj + residual for token chunk tci of half qt
                    hres = hress[qt]
                    for dt_ in range(2):
                        ps = psa_o([P, 512], "po")
                        for dcc in range(NDC // 2):
                            nc.tensor.matmul(
                                ps[:],
                                CTs[qt][:, 2 * dcc : 2 * dcc + 2, ts(tci, P)],
                                wo_sb[:, 2 * dcc : 2 * dcc + 2, ts(dt_, 512)],
                                start=(dcc == 0),
                                stop=(dcc == NDC // 2 - 1),
                                perf_mode=DR,
                            )
                        nc.sync.dma_start(
                            hres[:, tci, ts(dt_, 512)],
                            xloc_d[ts(qt * 4 + tci, P), ts(dt_, 512)],
                        )
                        nc.vector.tensor_add(
                            hres[:, tci, ts(dt_, 512)],
                            ps[:],
                            hres[:, tci, ts(dt_, 512)],
                        )
                def p3_ln_t(qt, tci, psa_t, hT_act):
                    # LN1 + transpose into hTs[qt]
                    row = hress[qt][:, tci, :]
                    layernorm_row(row, lnp, g1b, be1b, eps1_t, ln1_sqrt_scale)
                    for dc in range(NDC):
                        ps_t = psa_t([P, P], "pt")
                        nc.tensor.transpose(ps_t[:], row[:, ts(dc, P)], ident[:])
                        dst = hTs[qt][:, dc, ts(tci, P)]
                        if hT_act:
                            nc.scalar.activation(
                                dst, ps_t[:], AF.Identity,
                                scale=2.0 ** (SH - SLN),
                            )
                        else:
                            nc.vector.tensor_scalar_mul(
                                dst, ps_t[:], 2.0 ** (SH - SLN)
                            )

                def p3_chunk(qt, tci, psa_o, psa_t, hT_act):
                    p3_proj(qt, tci, psa_o)
                    p3_ln_t(qt, tci, psa_t, hT_act)

                def ffn1_chunk(qt, fc, uT, psa, relu_act, w1_sb=None,
                               colh=None):
                    if w1_sb is None:
                        w1_sb = w1p.tile([P, NDC, P], FP8, tag="w1")
                        nc.sync.dma_start(w1_sb[:], w1_r[:, :, ts(fc, P)])
                    cw = 512 if colh is None else 256
                    c0 = 0 if colh is None else colh * 256
                    ps = psa([P, cw], "pf")
                    for dcc in range(NDC // 2):
                        nc.tensor.matmul(
                            ps[:],
                            w1_sb[:, 2 * dcc : 2 * dcc + 2, :],
                            hTs[qt][:, 2 * dcc : 2 * dcc + 2, c0 : c0 + cw],
                            start=(dcc == 0),
                            stop=(dcc == NDC // 2 - 1),
                            perf_mode=DR,
                        )
                    # u = relu(ps + 2^6 b1); psum scale == s_u so no rescale
                    if relu_act:
                        nc.scalar.activation(
                            uT[:, fc, c0 : c0 + cw], ps[:], AF.Relu,
                            bias=b1c[:, fc : fc + 1],
                        )
                    else:
                        nc.vector.tensor_scalar(
                            out=uT[:, fc, c0 : c0 + cw],
                            in0=ps[:],
                            scalar1=b1c[:, fc : fc + 1],
                            scalar2=0.0,
                            op0=ALU.add,
                            op1=ALU.max,
                        )

                def ffn2_chunk(qt, tci, dt_, uT, psa):
                    # y = ps + hres written in place into hres (the row is
                    # dead as a residual once both halves are summed)
                    hres = hress[qt]
                    w2_sb = w2_sbs[dt_]
                    ps = psa([P, 512], "py")
                    for fcc in range(NFC // 2):
                        nc.tensor.matmul(
                            ps[:],
                            uT[:, 2 * fcc : 2 * fcc + 2, ts(tci, P)],
                            w2_sb[:, 2 * fcc : 2 * fcc + 2, :],
                            start=(fcc == 0),
                            stop=(fcc == NFC // 2 - 1),
                            perf_mode=DR,
                        )
                    nc.vector.tensor_add(
                        hres[:, tci, ts(dt_, 512)],
                        ps[:],
                        hres[:, tci, ts(dt_, 512)],
                    )
                    if dt_ == 1:
                        row = hres[:, tci, :]
                        if not ln_trivial:
                            nc.vector.tensor_add(row, row, b2b[:])
                        layernorm_row(row, lnp, g2b, be2b, eps2_t)
                        nc.gpsimd.dma_start(out_d[ts(qt * 4 + tci, P), :], row)

                with tc.tile_pool(name="qkvp", bufs=1) as qkvp:
                    QTs = [qkvp.tile([P, NDC, 512], FP8, name=f"QT{i}") for i in range(NQT)]
                    KTs = [qkvp.tile([P, NDC, 512], FP8, name=f"KT{i}") for i in range(NKT)]
                    VA = qkvp.tile([P, NKC, H, DK + 1], FP8)
                    nc.vector.memset(VA[:, :, :, DK : DK + 1], 1.0)
                    wv_sb = qkvp.tile([P, NDC, D], FP8)
                    wk_sb = qkvp.tile([P, NDC, D], FP8)
                    if DEFER_K:
                        wq_sb = qkvp.tile([P, NDC, D], FP8)

                    # ---- phase 1: Q/K projections (xt streamed) -----------
                    with (
                        tc.tile_pool(name="p1w", bufs=1) as p1w,
                        tc.tile_pool(name="p1x", bufs=(8 if ln_trivial else 6)) as p1x,
                        tc.tile_pool(name="p1ps", bufs=4, space="PSUM") as p1ps,
                    ):
                        if not DEFER_K:
                            wq_sb = p1w.tile([P, NDC, D], FP8, tag="w",
                                             name="wq_sb")
                        nc.sync.dma_start(
                            wq_sb[:, 0:2, :], wq_r[:, 0:2, :]
                        )
                        xss0 = []
                        for dcc in range(NDC // 2):
                            xs = p1x.tile([P, 2, 512], FP8, tag="xs")
                            nc.sync.dma_start(
                                xs[:], xt_r[:, 2 * dcc : 2 * dcc + 2, ts(0, 512)]
                            )
                            xss0.append(xs)
                        for dcc in range(1, NDC // 2):
                            nc.sync.dma_start(
                                wq_sb[:, 2 * dcc : 2 * dcc + 2, :],
                                wq_r[:, 2 * dcc : 2 * dcc + 2, :],
                            )
                        for dcc in range(NDC // 2):
                            nc.sync.dma_start(
                                wk_sb[:, 2 * dcc : 2 * dcc + 2, :],
                                wk_r[:, 2 * dcc : 2 * dcc + 2, :],
                            )
                        nc.gpsimd.dma_start(wv_sb[:], wv_r)
                        for kt in range(NKT):
                            if kt == 0:
                                xss = xss0
                            else:
                                xss = []
                                for dcc in range(NDC // 2):
                                    xs = p1x.tile([P, 2, 512], FP8, tag="xs")
                                    nc.sync.dma_start(
                                        xs[:],
                                        xt_r[:, 2 * dcc : 2 * dcc + 2, ts(kt, 512)],
                                    )
                                    xss.append(xs)
                            jobs = [(wk_sb, bkc, KTs[kt])]
                            if kt == 0 or (kt < NQT and not DEFER_K):
                                jobs.insert(0, (wq_sb, bqc, QTs[kt]))
                            if kt >= NQT and DEFER_K:
                                continue
                            for w_sb, bias_c, dst in jobs:
                                # two output groups of 4 psum accumulators
                                for g in range(2):
                                    pss = [
                                        p1ps.tile([P, 512], F32, tag="pqk",
                                                  name="pqk")
                                        for _ in range(4)
                                    ]
                                    for dcc in range(NDC // 2):
                                        for i in range(4):
                                            dc_out = g * 4 + i
                                            nc.tensor.matmul(
                                                pss[i][:],
                                                w_sb[:, 2 * dcc : 2 * dcc + 2,
                                                     ts(dc_out, P)],
                                                xss[dcc][:],
                                                start=(dcc == 0),
                                                stop=(dcc == NDC // 2 - 1),
                                                perf_mode=DR,
                                            )
                                    for i in range(4):
                                        if g == 0:
                                            nc.scalar.activation(
                                                dst[:, g * 4 + i, :],
                                                pss[i][:],
                                                AF.Identity,
                                                bias=bias_c[:, g * 4 + i : g * 4 + i + 1],
                                            )
                                        else:
                                            nc.vector.tensor_scalar_add(
                                                dst[:, g * 4 + i, :],
                                                pss[i][:],
                                                bias_c[:, g * 4 + i : g * 4 + i + 1],
                                            )


                    # ---- attention + injected V/p3/ffn fillers ------------
                    with (
                        tc.tile_pool(name="expp", bufs=(3 if ln_trivial else 2)) as expp,
                        tc.tile_pool(name="recp", bufs=2) as recp,
                        tc.tile_pool(name="vp", bufs=1) as vp,
                        tc.tile_pool(name="psS", bufs=2, space="PSUM") as psS,
                        tc.tile_pool(name="psC", bufs=2, space="PSUM") as psC,
                        tc.tile_pool(name="psF", bufs=2, space="PSUM") as psF,
                    ):
                        def psa_f(shape, name):
                            return psF.tile(shape, F32, tag="pw", name=name)

                        def k_tail(kt):
                            # deferred K tiles kt=2,3: psums borrow the (yet
                            # unused) ctx accumulator slots; DVE evicts keep
                            # the ACT queue pure exp
                            xss = []
                            for dcc in range(NDC // 2):
                                xs = vp.tile([P, 2, 512], FP8, tag="xs2",
                                             bufs=6)
                                nc.sync.dma_start(
                                    xs[:],
                                    xt_r[:, 2 * dcc : 2 * dcc + 2, ts(kt, 512)],
                                )
                                xss.append(xs)
                            for g in range(4):
                                pss = [
                                    psC.tile([P, 512], F32, tag="ps_cA",
                                             name="pkA", bufs=1),
                                    psC.tile([P, 512], F32, tag="ps_cB",
                                             name="pkB", bufs=1),
                                ]
                                for dcc in range(NDC // 2):
                                    for i in range(2):
                                        dc_out = g * 2 + i
                                        nc.tensor.matmul(
                                            pss[i][:],
                                            wk_sb[:, 2 * dcc : 2 * dcc + 2,
                                                  ts(dc_out, P)],
                                            xss[dcc][:],
                                            start=(dcc == 0),
                                            stop=(dcc == NDC // 2 - 1),
                                            perf_mode=DR,
                                        )
                                for i in range(2):
                                    dc_out = g * 2 + i
                                    nc.vector.tensor_scalar_add(
                                        KTs[kt][:, dc_out, :],
                                        pss[i][:],
                                        bkc[:, dc_out : dc_out + 1],
                                    )

                        def q1_tail():
                            # deferred Q tile kt=1 (first needed at unit 16)
                            xss = []
                            for dcc in range(NDC // 2):
                                xs = vp.tile([P, 2, 512], FP8, tag="xs2",
                                             bufs=6)
                                nc.sync.dma_start(
                                    xs[:],
                                    xt_r[:, 2 * dcc : 2 * dcc + 2, ts(1, 512)],
                                )
                                xss.append(xs)
                            for g in range(4):
                                pss = [psa_f([P, 512], "pq") for _ in range(2)]
                                for dcc in range(NDC // 2):
                                    for i in range(2):
                                        dc_out = g * 2 + i
                                        nc.tensor.matmul(
                                            pss[i][:],
                                            wq_sb[:, 2 * dcc : 2 * dcc + 2,
                                                  ts(dc_out, P)],
                                            xss[dcc][:],
                                            start=(dcc == 0),
                                            stop=(dcc == NDC // 2 - 1),
                                            perf_mode=DR,
                                        )
                                for i in range(2):
                                    dc_out = g * 2 + i
                                    nc.vector.tensor_scalar_add(
                                        QTs[1][:, dc_out, :],
                                        pss[i][:],
                                        bqc[:, dc_out : dc_out + 1],
                                    )

                        def v_chunks(kc0):
                            # V projection for key chunks kc0..kc0+7 (evicts
                            # into VA); psums use the filler ring
                            for kc in range(kc0, kc0 + 8):
                                xv = vp.tile([P, NDC, P], FP8, tag="xv", bufs=2)
                                nc.sync.dma_start(xv[:], xt_r[:, :, ts(kc, P)])
                                for dt_ in range(2):
                                    ps = psa_f([P, 512], "pv")
                                    for dcc in range(NDC // 2):
                                        nc.tensor.matmul(
                                            ps[:],
                                            xv[:, 2 * dcc : 2 * dcc + 2, :],
                                            wv_sb[:, 2 * dcc : 2 * dcc + 2,
                                                  ts(dt_, 512)],
                                            start=(dcc == 0),
                                            stop=(dcc == NDC // 2 - 1),
                                            perf_mode=DR,
                                        )
                                    nc.vector.tensor_copy(
                                        VA[:, kc, dt_ * 8 : (dt_ + 1) * 8, 0:DK],
                                        ps[:].rearrange("p (h d) -> p h d", h=8),
                                    )

                        cur_pc = {}

                        def ctx_ops(hc, qt, half):
                            if half == 0:
                                pcA = psC.tile([P, 512], F32, tag="ps_cA",
                                               name="ps_cA", bufs=1)
                                pcB = psC.tile([P, 512], F32, tag="ps_cB",
                                               name="ps_cB", bufs=1)
                                cur_pc[(hc, qt)] = (pcA, pcB)
                            pcA, pcB = cur_pc[(hc, qt)]
                            ops = []
                            for j, pc in enumerate((pcA, pcB)):
                                h = 2 * hc + j
                                for k4 in range(4):
                                    ops.append((pc, h, half, k4, j))
                            return ops

                        def emit_ctx_mm(op, et):
                            # DoubleRow over a kc pair: VA[:, kcpair, h, :]
                            # [128,2,65] x et[:, i8 pair, j, :] [128,2,512]
                            pc, h, half, k4, j = op
                            kc0 = half * 8 + 2 * k4
                            nc.tensor.matmul(
                                pc[0 : DK + 1, :],
                                VA[:, kc0 : kc0 + 2, h, :],
                                et[:, 2 * k4 : 2 * k4 + 2, j, :],
                                start=(kc0 == 0),
                                stop=(kc0 == NKC - 2),
                                perf_mode=DR,
                            )

                        def emit_recip_tail(hc, qt):
                            pcA, pcB = cur_pc.pop((hc, qt))
                            for j, pc in enumerate((pcA, pcB)):
                                rec = recp.tile([1, 512], F32R, tag="rec",
                                                name="rec", bufs=1)
                                with nc.allow_low_precision(reason="f32r"):
                                    nc.vector.reciprocal(
                                        rec[:], pc[DK : DK + 1, :]
                                    )
                                # evict ctx to bf16 SBUF immediately so the
                                # psum accumulator frees; bufs=1 accumulators
                                # then suffice
                                ctr = recp.tile([DK, 512], BF16, tag="ctr",
                                                name="ctr")
                                nc.vector.tensor_copy(ctr[:], pc[0:DK, :])
                                ps_b = psF.tile([DK, 512], F32, tag="pw",
                                                name="ps_b2")
                                nc.tensor.matmul(
                                    ps_b[:], ones_r[:], rec[:],
                                    start=True, stop=True,
                                )
                                nc.vector.tensor_mul(
                                    CTs[qt][DK * j : DK * j + DK, hc, :],
                                    ctr[:],
                                    ps_b[:],
                                )

                        def w2b_fetch():
                            nc.gpsimd.dma_start(
                                w2_sbs[1][:], w2_r[:, :, ts(1, 512)]
                            )

                        # filler schedule: (after_unit_idx, emitter). V lands
                        # under the first units; all qt0 p3/ffn work runs
                        # under the qt1 attention window.
                        def wo_fetch():
                            nc.gpsimd.dma_start(wo_sb[:], wo_r)
                            if _rep == 0:
                                late_const_dmas()

                        def w2a_fetch():
                            nc.gpsimd.dma_start(
                                w2_sbs[0][:], w2_r[:, :, 0:512]
                            )

                        fillers = ([
                            (0, lambda: k_tail(2)),
                            (0, lambda: k_tail(3)),
                            (2, q1_tail),
                        ] if DEFER_K else []) + [
                            (0, lambda: v_chunks(0)),
                            (1, lambda: v_chunks(8)),
                            (3, wo_fetch),
                            (8, w2a_fetch),
                            (17, w2b_fetch),
                        ]
                        for i in range(4):
                            fillers.append(
                                (16 + i,
                                 lambda t=i: p3_chunk(0, t, psa_f, psa_f, False)))
                        for fc in range(NFC):
                            fillers.append(
                                (20 + fc // 4,
                                 lambda f=fc: ffn1_chunk(0, f, uT0, psa_f, False)))
                        fi = 0
                        for tci in range(4):
                            for dt_ in range(2):
                                fillers.append(
                                    (28 + fi // 2,
                                     lambda t=tci, d=dt_: ffn2_chunk(0, t, d, uT0, psa_f)))
                                fi += 1
                        fillers.sort(key=lambda p: p[0])

                        units = [(hc, qt, half)
                                 for qt in range(NQT)
                                 for hc in range(H // 2)
                                 for half in (0, 1)]
                        prev = None
                        fidx = 0
                        for ui, u in enumerate(units):
                            hc, qt, half = u
                            pops = ctx_ops(*prev[0]) if prev else []
                            pet = prev[1] if prev else None
                            pidx = 0
                            et = expp.tile([P, 8, 2, 512], FP8, tag="exp",
                                           name="exph")
                            for i8 in range(8):
                                kc = half * 8 + i8
                                ps_s = psS.tile([P, 2, 512], F32, tag="ps_s",
                                                name="ps_s")
                                for j in range(2):
                                    p0 = DK * j
                                    nc.tensor.matmul(
                                        ps_s[:, j, :],
                                        KTs[kc // 4][p0 : p0 + DK, hc,
                                                     ts(kc % 4, P)],
                                        QTs[qt][p0 : p0 + DK, hc, :],
                                        start=True,
                                        stop=True,
                                    )
                                nc.scalar.activation(
                                    et[:, i8, :, :], ps_s[:],
                                    AF.Exp, scale=0.125 * 2.0 ** (-2 * SQ),
                                )
                                if pidx < len(pops):
                                    emit_ctx_mm(pops[pidx], pet)
                                    pidx += 1
                            while pidx < len(pops):
                                emit_ctx_mm(pops[pidx], pet)
                                pidx += 1
                            if prev is not None and prev[0][2] == 1:
                                emit_recip_tail(prev[0][0], prev[0][1])
                            prev = (u, et)
                            while fidx < len(fillers) and fillers[fidx][0] <= ui:
                                fillers[fidx][1]()
                                fidx += 1
                        for op in ctx_ops(*prev[0]):
                            emit_ctx_mm(op, prev[1])
                        emit_recip_tail(prev[0][0], prev[0][1])
                        while fidx < len(fillers):
                            fillers[fidx][1]()
                            fidx += 1

                # ---- tail: qt1 p3 + FFN1 + FFN2 (qkv pools freed) ---------
                with (
                    tc.tile_pool(name="tailp", bufs=1) as tailp,
                    tc.tile_pool(name="psZ", bufs=8, space="PSUM") as psZ,
                ):
                    uT1 = tailp.tile([P, NFC, 512], FP8)

                    def psa_z(shape, name):
                        return psZ.tile(shape, F32, tag="pz", name=name)

                    for tci in range(4):
                        p3_proj(1, tci, psa_z)
                    p3_ln_t(1, 0, psa_z, True)
                    p3_ln_t(1, 1, psa_z, True)
                    w1ts = []
                    for fc in range(NFC):
                        w1t = tailp.tile([P, NDC, P], FP8, name=f"w1t{fc}")
                        nc.gpsimd.dma_start(w1t[:], w1_r[:, :, ts(fc, P)])
                        w1ts.append(w1t)
                    # FFN1 on token cols 0-255 (chunks 0,1) while p3 finishes
                    # the rest; relu alternates ACT/DVE to split the load
                    for fc in range(NFC):
                        ffn1_chunk(1, fc, uT1, psa_z, fc % 2 == 0, w1ts[fc],
                                   colh=0)
                        if fc == 3:
                            p3_ln_t(1, 2, psa_z, True)
                        if fc == 11:
                            p3_ln_t(1, 3, psa_z, True)
                    for fc in range(NFC):
                        ffn1_chunk(1, fc, uT1, psa_z, fc % 2 == 0, w1ts[fc],
                                   colh=1)
                        # tc0/tc1 read only uT1 cols 0-255 (all colh0): their
                        # FFN2 groups can start as soon as colh1 begins
                        if fc == 0:
                            ffn2_chunk(1, 0, 0, uT1, psa_z)
                        if fc == 4:
                            ffn2_chunk(1, 0, 1, uT1, psa_z)
                        if fc == 8:
                            ffn2_chunk(1, 1, 0, uT1, psa_z)
                        if fc == 12:
                            ffn2_chunk(1, 1, 1, uT1, psa_z)
                    for tci in range(2, 4):
                        for dt_ in range(2):
                            ffn2_chunk(1, tci, dt_, uT1, psa_z)

    if waitfix:
        fix_multiwait(nc)
    return nc


# ---------------------------------------------------------------------------
def prepare_in_maps(x, mask, Wq, bq, Wk, bk, Wv, bv, Wo, bo, W1, b1, W2, b2,
                    g1, be1, g2, be2):
    f8 = ml_dtypes.float8_e4m3fn
    x = np.asarray(x, np.float32)
    Wo32 = np.asarray(Wo, np.float32)
    bo_eff = np.asarray(bo, np.float32) + np.asarray(bv, np.float32) @ Wo32

    def q8(a, s):  # scale by 2^s then quantize e4m3
        return np.ascontiguousarray(
            (np.asarray(a, np.float32) * 2.0 ** s).astype(f8)
        )

    def col(b_, n, s=0):  # [n*128] -> [128, n] column layout
        return np.ascontiguousarray(
            (np.asarray(b_, np.float32) * 2.0 ** s).reshape(n, P).T
        )

    def row(b_, s=0):
        return np.ascontiguousarray(
            (np.asarray(b_, np.float32) * 2.0 ** s).reshape(1, -1)
        )

    shared = {
        "wq": q8(Wq, SW),
        "wk": q8(Wk, SW),
        "wv": q8(Wv, SW),
        "wo": q8(Wo32, SW),
        "w1": q8(W1, SW1),
        "w2": q8(W2, SW2),
        "bqc": col(bq, NDC, SQ),
        "bkc": col(bk, NDC, SQ),
        "b1c": col(b1, NFC, SU),
        "b2r": row(b2, SLN),
        "g1r": row(g1, SLN),
        "be1r": row(be1, SLN),
        "g2r": row(g2),
        "be2r": row(be2),
    }

    in_maps = []
    for c in range(8):
        b_, hf = c // 2, c % 2
        xb = x[b_]  # [2048, 1024]
        loc = xb[hf * S_LOC : (hf + 1) * S_LOC, :]
        rem = xb[(1 - hf) * S_LOC : (2 - hf) * S_LOC, :]
        m = dict(shared)
        # token axis rolled: local tokens first (keys are permutation-inv.)
        m["xt"] = np.ascontiguousarray(
            (np.concatenate([loc, rem], axis=0).T * 2.0 ** SX).astype(f8)
        )
        m["xloc"] = np.ascontiguousarray(
            (loc + bo_eff[None, :]) * 2.0 ** SRES
        )
        in_maps.append(m)
    return in_maps


_NC = {}
LAST_RESULTS = None  # BassKernelResults of the most recent kernel() call


def detect_ln_trivial(g1, be1, g2, be2, b2, **_):
    return bool(
        np.all(np.asarray(g1) == 1.0) and np.all(np.asarray(be1) == 0.0)
        and np.all(np.asarray(g2) == 1.0) and np.all(np.asarray(be2) == 0.0)
        and np.all(np.asarray(b2) == 0.0)
    )


def kernel(x, mask, Wq, bq, Wk, bk, Wv, bv, Wo, bo, W1, b1, W2, b2, g1, be1, g2, be2):
    triv = detect_ln_trivial(g1=g1, be1=be1, g2=g2, be2=be2, b2=b2)
    if triv not in _NC:
        _NC[triv] = build_program(ln_trivial=triv)
    nc = _NC[triv]

    in_maps = prepare_in_maps(x, mask, Wq, bq, Wk, bk, Wv, bv, Wo, bo,
                              W1, b1, W2, b2, g1, be1, g2, be2)

    res = run_bass_kernel_spmd(nc, in_maps, list(range(8)))
    global LAST_RESULTS
    LAST_RESULTS = res

    out = np.empty((4, S_FULL, D), np.float32)
    for c in range(8):
        b_, hf = c // 2, c % 2
        out[b_, hf * S_LOC : (hf + 1) * S_LOC, :] = res.results[c]["out"]
    return out


# revision 61
# speedup vs baseline: 1.0432x; 1.0432x over previous
"""Trainium2 Bass kernel for a post-LN transformer encoder layer.

Reference computation (fp32, per batch b):
    q,k,v = x@Wq+bq, x@Wk+bk, x@Wv+bv          (D=1024, H=16 heads, dk=64)
    attn  = softmax(q k^T / sqrt(dk)) v         (S=2048, mask is all-ones)
    h     = LN(x + attn@Wo + bo; g1, be1)
    out   = LN(h + relu(h@W1+b1)@W2 + b2; g2, be2)

Sharding: 8 cores, fully independent (no collectives). Core c owns batch
b=c//2, sequence half c%2 (1024 query tokens), and redundantly computes
K/V for its full batch (2048 keys) from a host-provided transposed copy
of x. The host rolls the token axis so each core's local tokens come
first (attention is permutation-invariant over keys).

fp8 scheme: all big matmuls run fp8e4m3 in DoubleRow perf mode (0.5
cycles/row = 2x PE throughput; scores stay normal-mode fp8 since their
contraction dim dk=64 lives on partitions). Operands carry power-of-2
scales chosen so every eviction needs at most a bias add (no ACT scale):
s_x*s_w == s_q and s_w1*s_h == s_u, letting Q/K land via ACT
Identity+bias, V via DVE copy, and relu via DVE (ps+bias max 0). The
layernorm outputs absorb the residual scales (LN is invariant to input
scaling once eps is scaled to match). Exp tiles are raw exp values in
fp8 (score sigma is ~0.33 after the 1/sqrt(dk) fold, so exp in
[~0.25, ~4] fits e4m3); the softmax denominator rides as an all-ones V
column so et/V scales cancel exactly in ctx/denom.

Scales (log2): x:2 W(qkvo):4 q/k/v/ctx:6 hres:10 h(postLN1):15 hT:1
W1:5 u:6 W2:9; ffn psum 6+9=15 matches h; eps1=1e-5*2^20 eps2=1e-5*2^30.

Schedule (emission order = per-engine queue order; Tile adds the deps):
  p1: Q,K (DoubleRow, ACT evict, two 4-psum output groups) then V
      (DoubleRow, DVE evict + ones col); w2 half A prefetch.
  attention: paired-head units (hc,qt,half), qt-major, software-pipelined
    scores (fp8, K=64) -> exp (ACT, fp8 out) -> ctx (DoubleRow kc-pairs)
    -> recip tail (DVE recip + PE ones-bcast + DVE mul -> CT fp8).
    Filler closures are injected into late units so PE/DVE gaps under the
    ACT-bound exp stream do useful work: p3 for qt0 tokens (out-proj
    DoubleRow + residual + LN1 + PE-transpose -> hT), FFN1 qt0
    (DoubleRow + DVE relu -> uT), FFN2 + LN2 + output DMA for qt0 tokens.
  tail: p3 qt1 -> hT, FFN1 qt1, FFN2 + LN2 for qt1 tokens.
ACT's in-order queue stays essentially pure exp; LN rsqrt ops are emitted
right after a unit's exp burst so their DVE inputs are ready when reached.
"""

import numpy as np
import ml_dtypes

import concourse.bass as bass
import concourse.mybir as mybir
import concourse.tile as tile
from concourse.bass import ts
from concourse.bass_utils import run_bass_kernel_spmd
from concourse.masks import make_identity

BF16 = mybir.dt.bfloat16
F32 = mybir.dt.float32
F32R = mybir.dt.float32r
FP8 = mybir.dt.float8e4
AF = mybir.ActivationFunctionType
ALU = mybir.AluOpType
DR = mybir.MatmulPerfMode.DoubleRow

D = 1024
DFF = 4096
H = 16
DK = 64
S_FULL = 2048
S_LOC = 1024
P = 128
NDC = D // P        # 8  feature chunks
NFC = DFF // P      # 32 ffn chunks
NKC = S_FULL // P   # 16 key chunks
NTC = S_LOC // P    # 8  local token chunks
NQT = S_LOC // 512  # 2 query tiles of 512
NKT = S_FULL // 512 # 4 key-token tiles of 512

# scales (log2); SX+SW == SQ(==SV) and SW1+SH == SU so evictions skip scales
SX = 2
SW = 4
SQ = 6
SV = 6
SW1 = 5
SH = 1
SU = 6
SW2 = 9
SRES = SV + SW   # 10: hres scale pre-LN1 (ctx@Wo psum scale)
SLN = SU + SW2   # 15: post-LN1 h scale == FFN2 psum scale
EPS1 = 1e-5 * 2.0 ** (2 * SRES)
EPS2 = 1e-5 * 2.0 ** (2 * SLN)


# ---------------------------------------------------------------------------
# Multi-wait splitting: this walrus build rejects instructions carrying more
# than one sync-wait command. Tile occasionally emits several (notably the
# kernel-tail drain). Keep the last wait on the instruction and hoist the
# rest onto NoOps inserted just before it on the same engine queue.
_ctr = [0]


def _split_block(bb):
    out = []
    changed = False
    for inst in bb.instructions:
        si = inst.sync_info
        waits = list(si.on_wait) if si is not None and si.on_wait else []
        if len(waits) > 1:
            changed = True
            for w in waits[:-1]:
                _ctr[0] += 1
                nop = mybir.InstNoOp(name=f"waitfix-{_ctr[0]}", ins=[], outs=[])
                nop.engine = inst.engine
                nop.sync_info = mybir.SyncInfo(on_wait=[w], on_update=[])
                out.append(nop)
            inst.sync_info = mybir.SyncInfo(
                on_wait=[waits[-1]], on_update=list(si.on_update or [])
            )
        out.append(inst)
    if changed:
        bb.instructions = out
    return changed


def fix_multiwait(nc):
    for fn in nc.m.functions:
        for bb in fn.blocks:
            _split_block(bb)


# ---------------------------------------------------------------------------
def build_program(reps=1, waitfix=True, ln_trivial=False):
    # the deferred Q/K fillers need extra vp buffers that only fit in the
    # ln_trivial layout (the gamma/beta broadcast tiles are dropped there)
    DEFER_K = ln_trivial
    nc = bass.Bass()

    xt_d = nc.dram_tensor("xt", [D, S_FULL], FP8, kind="ExternalInput")
    xloc_d = nc.dram_tensor("xloc", [S_LOC, D], F32, kind="ExternalInput")
    wq_d = nc.dram_tensor("wq", [D, D], FP8, kind="ExternalInput")
    wk_d = nc.dram_tensor("wk", [D, D], FP8, kind="ExternalInput")
    wv_d = nc.dram_tensor("wv", [D, D], FP8, kind="ExternalInput")
    wo_d = nc.dram_tensor("wo", [D, D], FP8, kind="ExternalInput")
    w1_d = nc.dram_tensor("w1", [D, DFF], FP8, kind="ExternalInput")
    w2_d = nc.dram_tensor("w2", [DFF, D], FP8, kind="ExternalInput")
    bqc_d = nc.dram_tensor("bqc", [P, NDC], F32, kind="ExternalInput")
    bkc_d = nc.dram_tensor("bkc", [P, NDC], F32, kind="ExternalInput")
    b1c_d = nc.dram_tensor("b1c", [P, NFC], F32, kind="ExternalInput")
    b2r_d = nc.dram_tensor("b2r", [1, D], F32, kind="ExternalInput")
    g1r_d = nc.dram_tensor("g1r", [1, D], F32, kind="ExternalInput")
    be1r_d = nc.dram_tensor("be1r", [1, D], F32, kind="ExternalInput")
    g2r_d = nc.dram_tensor("g2r", [1, D], F32, kind="ExternalInput")
    be2r_d = nc.dram_tensor("be2r", [1, D], F32, kind="ExternalInput")
    out_d = nc.dram_tensor("out", [S_LOC, D], F32, kind="ExternalOutput")

    xt_r = xt_d.rearrange("(dc p) t -> p dc t", p=P)
    wq_r = wq_d.rearrange("(dc p) o -> p dc o", p=P)
    wk_r = wk_d.rearrange("(dc p) o -> p dc o", p=P)
    wv_r = wv_d.rearrange("(dc p) o -> p dc o", p=P)
    wo_r = wo_d.rearrange("(dc p) o -> p dc o", p=P)
    w1_r = w1_d.rearrange("(dc p) f -> p dc f", p=P)
    w2_r = w2_d.rearrange("(fc p) o -> p fc o", p=P)

    def bcast_row(row_d):
        # [1, D] dram row -> partition-broadcast AP for DMA into [P, D]
        a = row_d[0:1, :]
        return bass.AP(tensor=a.tensor, offset=a.offset, ap=[[0, P], [1, D]])

    def layernorm_row(row, lnp, g_b, be_b, eps_t, sqrt_scale=1.0):
        # With ln_trivial (gamma==1, beta==0 detected host-side), the output
        # rescale folds into the Sqrt activation scale: r = recip(sqrt(
        # sqrt_scale*var + bias)) makes (row-mu)*r directly the desired
        # output, and the gamma/beta tensor ops are skipped entirely.
        st = lnp.tile([P, 2, 6], F32, tag="st")
        nc.vector.bn_stats(st[:, 0, :], row[:, 0:512])
        nc.vector.bn_stats(st[:, 1, :], row[:, 512:1024])
        mv = lnp.tile([P, 2], F32, tag="mv")
        nc.vector.bn_aggr(mv[:], st[:])
        nc.scalar.activation(mv[:, 1:2], mv[:, 1:2], AF.Sqrt, bias=eps_t[:],
                             scale=sqrt_scale)
        nc.vector.reciprocal(mv[:, 1:2], mv[:, 1:2])
        nc.vector.tensor_scalar(
            out=row,
            in0=row,
            scalar1=mv[:, 0:1],
            scalar2=mv[:, 1:2],
            op0=ALU.subtract,
            op1=ALU.mult,
        )
        if not ln_trivial:
            nc.vector.tensor_mul(row, row, g_b[:])
            nc.vector.tensor_add(row, row, be_b[:])

    with tile.TileContext(nc) as tc:
        with (
            tc.tile_pool(name="top", bufs=1) as top,
            tc.tile_pool(name="lnp", bufs=2) as lnp,
        ):
            # ---- whole-kernel constants -----------------------------------
            ident = top.tile([P, P], F32)
            make_identity(nc, ident)
            eps1_t = top.tile([P, 1], F32)
            eps2_t = top.tile([P, 1], F32)
            nc.vector.memset(eps2_t, EPS2)
            if ln_trivial:
                # LN1 sqrt computes 2^-SLN+SRES... r folds the 2^SLN output
                # scale: sqrt(2^(-2SLN)*var_s + 1e-5*2^(2SRES-2SLN))
                nc.vector.memset(eps1_t, 1e-5 * 2.0 ** (2 * SRES - 2 * SLN))
                ln1_sqrt_scale = 2.0 ** (-2 * SLN)
            else:
                nc.vector.memset(eps1_t, EPS1)
                ln1_sqrt_scale = 1.0
            ones32 = top.tile([1, DK], F32)
            nc.vector.memset(ones32, 1.0)
            ones_r = top.tile([1, DK], F32R)
            with nc.allow_low_precision(reason="f32r round for PE broadcast"):
                nc.vector.tensor_copy(ones_r[:], ones32[:])
            bqc = top.tile([P, NDC], F32)
            nc.sync.dma_start(bqc[:], bqc_d[:])
            bkc = top.tile([P, NDC], F32)
            nc.sync.dma_start(bkc[:], bkc_d[:])
            b1c = top.tile([P, NFC], F32)
            if ln_trivial:
                b2b = g1b = be1b = g2b = be2b = None

                def late_const_dmas():
                    nc.gpsimd.dma_start(b1c[:], b1c_d[:])
            else:
                b2b = top.tile([P, D], F32)
                g1b = top.tile([P, D], F32)
                be1b = top.tile([P, D], F32)
                g2b = top.tile([P, D], F32)
                be2b = top.tile([P, D], F32)

                def late_const_dmas():
                    nc.gpsimd.dma_start(b1c[:], b1c_d[:])
                    nc.gpsimd.dma_start(b2b[:], bcast_row(b2r_d))
                    nc.gpsimd.dma_start(g1b[:], bcast_row(g1r_d))
                    nc.gpsimd.dma_start(be1b[:], bcast_row(be1r_d))
                    nc.gpsimd.dma_start(g2b[:], bcast_row(g2r_d))
                    nc.gpsimd.dma_start(be2b[:], bcast_row(be2r_d))

            for _rep in range(reps):
              with (
                tc.tile_pool(name="repp", bufs=1) as repp,
                tc.tile_pool(name="w1p", bufs=2) as w1p,
                tc.tile_pool(name="wop", bufs=1) as wop,
              ):
                # persistents that cross the attention/tail boundary
                CTs = [repp.tile([P, NDC, 512], FP8, name=f"CT{i}") for i in range(NQT)]
                hress = [repp.tile([P, 4, D], F32, name=f"hres{i}") for i in range(NQT)]
                hTs = [repp.tile([P, NDC, 512], FP8, name=f"hT{i}") for i in range(NQT)]
                uT0 = repp.tile([P, NFC, 512], FP8)
                w2_sbs = [repp.tile([P, NFC, 512], FP8, name=f"w2h{i}") for i in range(2)]
                wo_sb = wop.tile([P, NDC, D], FP8)

                # ---- chunk emitters (fillers during attention + tail) -----
                # psa(shape, name) allocates a psum tile from the active ring
                def p3_proj(qt, tci, psa_o):
                    # out-proj + residual for token chunk tci of half qt
                    hres = hress[qt]
                    for dt_ in range(2):
                        ps = psa_o([P, 512], "po")
                        for dcc in range(NDC // 2):
                            nc.tensor.matmul(
                                ps[:],
                                CTs[qt][:, 2 * dcc : 2 * dcc + 2, ts(tci, P)],
                                wo_sb[:, 2 * dcc : 2 * dcc + 2, ts(dt_, 512)],
                                start=(dcc == 0),
                                stop=(dcc == NDC // 2 - 1),
                                perf_mode=DR,
                            )
                        nc.sync.dma_start(
                            hres[:, tci, ts(dt_, 512)],
                            xloc_d[ts(qt * 4 + tci, P), ts(dt_, 512)],
                        )
                        nc.vector.tensor_add(
                            hres[:, tci, ts(dt_, 512)],
                            ps[:],
                            hres[:, tci, ts(dt_, 512)],
                        )
                def p3_ln_t(qt, tci, psa_t, hT_act):
                    # LN1 + transpose into hTs[qt]
                    row = hress[qt][:, tci, :]
                    layernorm_row(row, lnp, g1b, be1b, eps1_t, ln1_sqrt_scale)
                    for dc in range(NDC):
                        ps_t = psa_t([P, P], "pt")
                        nc.tensor.transpose(ps_t[:], row[:, ts(dc, P)], ident[:])
                        dst = hTs[qt][:, dc, ts(tci, P)]
                        if hT_act:
                            nc.scalar.activation(
                                dst, ps_t[:], AF.Identity,
                                scale=2.0 ** (SH - SLN),
                            )
                        else:
                            nc.vector.tensor_scalar_mul(
                                dst, ps_t[:], 2.0 ** (SH - SLN)
                            )

                def p3_chunk(qt, tci, psa_o, psa_t, hT_act):
                    p3_proj(qt, tci, psa_o)
                    p3_ln_t(qt, tci, psa_t, hT_act)

                def ffn1_chunk(qt, fc, uT, psa, relu_act, w1_sb=None,
                               colh=None):
                    if w1_sb is None:
                        w1_sb = w1p.tile([P, NDC, P], FP8, tag="w1")
                        nc.sync.dma_start(w1_sb[:], w1_r[:, :, ts(fc, P)])
                    cw = 512 if colh is None else 256
                    c0 = 0 if colh is None else colh * 256
                    ps = psa([P, cw], "pf")
                    for dcc in range(NDC // 2):
                        nc.tensor.matmul(
                            ps[:],
                            w1_sb[:, 2 * dcc : 2 * dcc + 2, :],
                            hTs[qt][:, 2 * dcc : 2 * dcc + 2, c0 : c0 + cw],
                            start=(dcc == 0),
                            stop=(dcc == NDC // 2 - 1),
                            perf_mode=DR,
                        )
                    # u = relu(ps + 2^6 b1); psum scale == s_u so no rescale
                    if relu_act:
                        nc.scalar.activation(
                            uT[:, fc, c0 : c0 + cw], ps[:], AF.Relu,
                            bias=b1c[:, fc : fc + 1],
                        )
                    else:
                        nc.vector.tensor_scalar(
                            out=uT[:, fc, c0 : c0 + cw],
                            in0=ps[:],
                            scalar1=b1c[:, fc : fc + 1],
                            scalar2=0.0,
                            op0=ALU.add,
                            op1=ALU.max,
                        )

                def ffn2_chunk(qt, tci, dt_, uT, psa):
                    # y = ps + hres written in place into hres (the row is
                    # dead as a residual once both halves are summed)
                    hres = hress[qt]
                    w2_sb = w2_sbs[dt_]
                    ps = psa([P, 512], "py")
                    for fcc in range(NFC // 2):
                        nc.tensor.matmul(
                            ps[:],
                            uT[:, 2 * fcc : 2 * fcc + 2, ts(tci, P)],
                            w2_sb[:, 2 * fcc : 2 * fcc + 2, :],
                            start=(fcc == 0),
                            stop=(fcc == NFC // 2 - 1),
                            perf_mode=DR,
                        )
                    nc.vector.tensor_add(
                        hres[:, tci, ts(dt_, 512)],
                        ps[:],
                        hres[:, tci, ts(dt_, 512)],
                    )
                    if dt_ == 1:
                        row = hres[:, tci, :]
                        if not ln_trivial:
                            nc.vector.tensor_add(row, row, b2b[:])
                        layernorm_row(row, lnp, g2b, be2b, eps2_t)
                        nc.gpsimd.dma_start(out_d[ts(qt * 4 + tci, P), :], row)

                with tc.tile_pool(name="qkvp", bufs=1) as qkvp:
                    QTs = [qkvp.tile([P, NDC, 512], FP8, name=f"QT{i}") for i in range(NQT)]
                    KTs = [qkvp.tile([P, NDC, 512], FP8, name=f"KT{i}") for i in range(NKT)]
                    VA = qkvp.tile([P, NKC, H, DK + 1], FP8)
                    nc.vector.memset(VA[:, :, :, DK : DK + 1], 1.0)
                    wv_sb = qkvp.tile([P, NDC, D], FP8)
                    wk_sb = qkvp.tile([P, NDC, D], FP8)
                    if DEFER_K:
                        wq_sb = qkvp.tile([P, NDC, D], FP8)

                    # ---- phase 1: Q/K projections (xt streamed) -----------
                    with (
                        tc.tile_pool(name="p1w", bufs=1) as p1w,
                        tc.tile_pool(name="p1x", bufs=(8 if ln_trivial else 6)) as p1x,
                        tc.tile_pool(name="p1ps", bufs=4, space="PSUM") as p1ps,
                    ):
                        if not DEFER_K:
                            wq_sb = p1w.tile([P, NDC, D], FP8, tag="w",
                                             name="wq_sb")
                        nc.sync.dma_start(
                            wq_sb[:, 0:2, :], wq_r[:, 0:2, :]
                        )
                        xss0 = []
                        for dcc in range(NDC // 2):
                            xs = p1x.tile([P, 2, 512], FP8, tag="xs")
                            nc.sync.dma_start(
                                xs[:], xt_r[:, 2 * dcc : 2 * dcc + 2, ts(0, 512)]
                            )
                            xss0.append(xs)
                        for dcc in range(1, NDC // 2):
                            nc.sync.dma_start(
                                wq_sb[:, 2 * dcc : 2 * dcc + 2, :],
                                wq_r[:, 2 * dcc : 2 * dcc + 2, :],
                            )
                        for dcc in range(NDC // 2):
                            nc.sync.dma_start(
                                wk_sb[:, 2 * dcc : 2 * dcc + 2, :],
                                wk_r[:, 2 * dcc : 2 * dcc + 2, :],
                            )
                        nc.gpsimd.dma_start(wv_sb[:], wv_r)
                        for kt in range(NKT):
                            if kt == 0:
                                xss = xss0
                            else:
                                xss = []
                                for dcc in range(NDC // 2):
                                    xs = p1x.tile([P, 2, 512], FP8, tag="xs")
                                    nc.sync.dma_start(
                                        xs[:],
                                        xt_r[:, 2 * dcc : 2 * dcc + 2, ts(kt, 512)],
                                    )
                                    xss.append(xs)
                            jobs = [(wk_sb, bkc, KTs[kt])]
                            if kt == 0 or (kt < NQT and not DEFER_K):
                                jobs.insert(0, (wq_sb, bqc, QTs[kt]))
                            if kt >= NQT and DEFER_K:
                                continue
                            for w_sb, bias_c, dst in jobs:
                                # two output groups of 4 psum accumulators
                                for g in range(2):
                                    pss = [
                                        p1ps.tile([P, 512], F32, tag="pqk",
                                                  name="pqk")
                                        for _ in range(4)
                                    ]
                                    for dcc in range(NDC // 2):
                                        for i in range(4):
                                            dc_out = g * 4 + i
                                            nc.tensor.matmul(
                                                pss[i][:],
                                                w_sb[:, 2 * dcc : 2 * dcc + 2,
                                                     ts(dc_out, P)],
                                                xss[dcc][:],
                                                start=(dcc == 0),
                                                stop=(dcc == NDC // 2 - 1),
                                                perf_mode=DR,
                                            )
                                    for i in range(4):
                                        if g == 0:
                                            nc.scalar.activation(
                                                dst[:, g * 4 + i, :],
                                                pss[i][:],
                                                AF.Identity,
                                                bias=bias_c[:, g * 4 + i : g * 4 + i + 1],
                                            )
                                        else:
                                            nc.vector.tensor_scalar_add(
                                                dst[:, g * 4 + i, :],
                                                pss[i][:],
                                                bias_c[:, g * 4 + i : g * 4 + i + 1],
                                            )


                    # ---- attention + injected V/p3/ffn fillers ------------
                    with (
                        tc.tile_pool(name="expp", bufs=(3 if ln_trivial else 2)) as expp,
                        tc.tile_pool(name="recp", bufs=2) as recp,
                        tc.tile_pool(name="vp", bufs=1) as vp,
                        tc.tile_pool(name="psS", bufs=2, space="PSUM") as psS,
                        tc.tile_pool(name="psC", bufs=2, space="PSUM") as psC,
                        tc.tile_pool(name="psF", bufs=2, space="PSUM") as psF,
                    ):
                        def psa_f(shape, name):
                            return psF.tile(shape, F32, tag="pw", name=name)

                        def k_tail(kt):
                            # deferred K tiles kt=2,3: psums borrow the (yet
                            # unused) ctx accumulator slots; DVE evicts keep
                            # the ACT queue pure exp
                            xss = []
                            for dcc in range(NDC // 2):
                                xs = vp.tile([P, 2, 512], FP8, tag="xs2",
                                             bufs=6)
                                nc.sync.dma_start(
                                    xs[:],
                                    xt_r[:, 2 * dcc : 2 * dcc + 2, ts(kt, 512)],
                                )
                                xss.append(xs)
                            for g in range(4):
                                pss = [
                                    psC.tile([P, 512], F32, tag="ps_cA",
                                             name="pkA", bufs=1),
                                    psC.tile([P, 512], F32, tag="ps_cB",
                                             name="pkB", bufs=1),
                                ]
                                for dcc in range(NDC // 2):
                                    for i in range(2):
                                        dc_out = g * 2 + i
                                        nc.tensor.matmul(
                                            pss[i][:],
                                            wk_sb[:, 2 * dcc : 2 * dcc + 2,
                                                  ts(dc_out, P)],
                                            xss[dcc][:],
                                            start=(dcc == 0),
                                            stop=(dcc == NDC // 2 - 1),
                                            perf_mode=DR,
                                        )
                                for i in range(2):
                                    dc_out = g * 2 + i
                                    nc.vector.tensor_scalar_add(
                                        KTs[kt][:, dc_out, :],
                                        pss[i][:],
                                        bkc[:, dc_out : dc_out + 1],
                                    )

                        def q1_tail():
                            # deferred Q tile kt=1 (first needed at unit 16)
                            xss = []
                            for dcc in range(NDC // 2):
                                xs = vp.tile([P, 2, 512], FP8, tag="xs2",
                                             bufs=6)
                                nc.sync.dma_start(
                                    xs[:],
                                    xt_r[:, 2 * dcc : 2 * dcc + 2, ts(1, 512)],
                                )
                                xss.append(xs)
                            for g in range(4):
                                pss = [psa_f([P, 512], "pq") for _ in range(2)]
                                for dcc in range(NDC // 2):
                                    for i in range(2):
                                        dc_out = g * 2 + i
                                        nc.tensor.matmul(
                                            pss[i][:],
                                            wq_sb[:, 2 * dcc : 2 * dcc + 2,
                                                  ts(dc_out, P)],
                                            xss[dcc][:],
                                            start=(dcc == 0),
                                            stop=(dcc == NDC // 2 - 1),
                                            perf_mode=DR,
                                        )
                                for i in range(2):
                                    dc_out = g * 2 + i
                                    nc.vector.tensor_scalar_add(
                                        QTs[1][:, dc_out, :],
                                        pss[i][:],
                                        bqc[:, dc_out : dc_out + 1],
                                    )

                        def v_chunks(kc0):
                            # V projection for key chunks kc0..kc0+7 (evicts
                            # into VA); psums use the filler ring
                            for kc in range(kc0, kc0 + 8):
                                xv = vp.tile([P, NDC, P], FP8, tag="xv", bufs=2)
                                nc.sync.dma_start(xv[:], xt_r[:, :, ts(kc, P)])
                                for dt_ in range(2):
                                    ps = psa_f([P, 512], "pv")
                                    for dcc in range(NDC // 2):
                                        nc.tensor.matmul(
                                            ps[:],
                                            xv[:, 2 * dcc : 2 * dcc + 2, :],
                                            wv_sb[:, 2 * dcc : 2 * dcc + 2,
                                                  ts(dt_, 512)],
                                            start=(dcc == 0),
                                            stop=(dcc == NDC // 2 - 1),
                                            perf_mode=DR,
                                        )
                                    nc.vector.tensor_copy(
                                        VA[:, kc, dt_ * 8 : (dt_ + 1) * 8, 0:DK],
                                        ps[:].rearrange("p (h d) -> p h d", h=8),
                                    )

                        cur_pc = {}

                        def ctx_ops(hc, qt, half):
                            if half == 0:
                                pcA = psC.tile([P, 512], F32, tag="ps_cA",
                                               name="ps_cA", bufs=1)
                                pcB = psC.tile([P, 512], F32, tag="ps_cB",
                                               name="ps_cB", bufs=1)
                                cur_pc[(hc, qt)] = (pcA, pcB)
                            pcA, pcB = cur_pc[(hc, qt)]
                            ops = []
                            for j, pc in enumerate((pcA, pcB)):
                                h = 2 * hc + j
                                for k4 in range(4):
                                    ops.append((pc, h, half, k4, j))
                            return ops

                        def emit_ctx_mm(op, et):
                            # DoubleRow over a kc pair: VA[:, kcpair, h, :]
                            # [128,2,65] x et[:, i8 pair, j, :] [128,2,512]
                            pc, h, half, k4, j = op
                            kc0 = half * 8 + 2 * k4
                            nc.tensor.matmul(
                                pc[0 : DK + 1, :],
                                VA[:, kc0 : kc0 + 2, h, :],
                                et[:, 2 * k4 : 2 * k4 + 2, j, :],
                                start=(kc0 == 0),
                                stop=(kc0 == NKC - 2),
                                perf_mode=DR,
                            )

                        def emit_recip_tail(hc, qt):
                            pcA, pcB = cur_pc.pop((hc, qt))
                            for j, pc in enumerate((pcA, pcB)):
                                rec = recp.tile([1, 512], F32R, tag="rec",
                                                name="rec", bufs=1)
                                with nc.allow_low_precision(reason="f32r"):
                                    nc.vector.reciprocal(
                                        rec[:], pc[DK : DK + 1, :]
                                    )
                                # evict ctx to bf16 SBUF immediately so the
                                # psum accumulator frees; bufs=1 accumulators
                                # then suffice
                                ctr = recp.tile([DK, 512], BF16, tag="ctr",
                                                name="ctr")
                                nc.vector.tensor_copy(ctr[:], pc[0:DK, :])
                                ps_b = psF.tile([DK, 512], F32, tag="pw",
                                                name="ps_b2")
                                nc.tensor.matmul(
                                    ps_b[:], ones_r[:], rec[:],
                                    start=True, stop=True,
                                )
                                nc.vector.tensor_mul(
                                    CTs[qt][DK * j : DK * j + DK, hc, :],
                                    ctr[:],
                                    ps_b[:],
                                )

                        def w2b_fetch():
                            nc.gpsimd.dma_start(
                                w2_sbs[1][:], w2_r[:, :, ts(1, 512)]
                            )

                        # filler schedule: (after_unit_idx, emitter). V lands
                        # under the first units; all qt0 p3/ffn work runs
                        # under the qt1 attention window.
                        def wo_fetch():
                            nc.gpsimd.dma_start(wo_sb[:], wo_r)
                            if _rep == 0:
                                late_const_dmas()

                        def w2a_fetch():
                            nc.gpsimd.dma_start(
                                w2_sbs[0][:], w2_r[:, :, 0:512]
                            )

                        fillers = ([
                            (0, lambda: k_tail(2)),
                            (0, lambda: k_tail(3)),
                            (2, q1_tail),
                        ] if DEFER_K else []) + [
                            (0, lambda: v_chunks(0)),
                            (1, lambda: v_chunks(8)),
                            (3, wo_fetch),
                            (8, w2a_fetch),
                            (17, w2b_fetch),
                        ]
                        for i in range(4):
                            fillers.append(
                                (16 + i,
                                 lambda t=i: p3_chunk(0, t, psa_f, psa_f, False)))
                        for fc in range(NFC):
                            fillers.append(
                                (20 + fc // 4,
                                 lambda f=fc: ffn1_chunk(0, f, uT0, psa_f, False)))
                        fi = 0
                        for tci in range(4):
                            for dt_ in range(2):
                                fillers.append(
                                    (28 + fi // 2,
                                     lambda t=tci, d=dt_: ffn2_chunk(0, t, d, uT0, psa_f)))
                                fi += 1
                        fillers.sort(key=lambda p: p[0])

                        units = [(hc, qt, half)
                                 for qt in range(NQT)
                                 for hc in range(H // 2)
                                 for half in (0, 1)]
                        prev = None
                        fidx = 0
                        for ui, u in enumerate(units):
                            hc, qt, half = u
                            pops = ctx_ops(*prev[0]) if prev else []
                            pet = prev[1] if prev else None
                            pidx = 0
                            et = expp.tile([P, 8, 2, 512], FP8, tag="exp",
                                           name="exph")
                            for i8 in range(8):
                                kc = half * 8 + i8
                                ps_s = psS.tile([P, 2, 512], F32, tag="ps_s",
                                                name="ps_s")
                                for j in range(2):
                                    p0 = DK * j
                                    nc.tensor.matmul(
                                        ps_s[:, j, :],
                                        KTs[kc // 4][p0 : p0 + DK, hc,
                                                     ts(kc % 4, P)],
                                        QTs[qt][p0 : p0 + DK, hc, :],
                                        start=True,
                                        stop=True,
                                    )
                                nc.scalar.activation(
                                    et[:, i8, :, :], ps_s[:],
                                    AF.Exp, scale=0.125 * 2.0 ** (-2 * SQ),
                                )
                                if pidx < len(pops):
                                    emit_ctx_mm(pops[pidx], pet)
                                    pidx += 1
                            while pidx < len(pops):
                                emit_ctx_mm(pops[pidx], pet)
                                pidx += 1
                            if prev is not None and prev[0][2] == 1:
                                emit_recip_tail(prev[0][0], prev[0][1])
                            prev = (u, et)
                            while fidx < len(fillers) and fillers[fidx][0] <= ui:
                                fillers[fidx][1]()
                                fidx += 1
                        for op in ctx_ops(*prev[0]):
                            emit_ctx_mm(op, prev[1])
                        emit_recip_tail(prev[0][0], prev[0][1])
                        while fidx < len(fillers):
                            fillers[fidx][1]()
                            fidx += 1

                # ---- tail: qt1 p3 + FFN1 + FFN2 (qkv pools freed) ---------
                with (
                    tc.tile_pool(name="tailp", bufs=1) as tailp,
                    tc.tile_pool(name="psZ", bufs=8, space="PSUM") as psZ,
                ):
                    uT1 = tailp.tile([P, NFC, 512], FP8)

                    def psa_z(shape, name):
                        return psZ.tile(shape, F32, tag="pz", name=name)

                    for tci in range(4):
                        p3_proj(1, tci, psa_z)
                    for tci in range(4):
                        p3_ln_t(1, tci, psa_z, True)
                    w1ts = []
                    for fc in range(NFC):
                        w1t = tailp.tile([P, NDC, P], FP8, name=f"w1t{fc}")
                        nc.gpsimd.dma_start(w1t[:], w1_r[:, :, ts(fc, P)])
                        w1ts.append(w1t)
                    # full-width FFN1; relu alternates ACT/DVE
                    for fc in range(NFC):
                        ffn1_chunk(1, fc, uT1, psa_z, fc % 2 == 0, w1ts[fc])
                    for tci in range(4):
                        for dt_ in range(2):
                            ffn2_chunk(1, tci, dt_, uT1, psa_z)

    if waitfix:
        fix_multiwait(nc)
    return nc


# ---------------------------------------------------------------------------
def prepare_in_maps(x, mask, Wq, bq, Wk, bk, Wv, bv, Wo, bo, W1, b1, W2, b2,
                    g1, be1, g2, be2):
    f8 = ml_dtypes.float8_e4m3fn
    x = np.asarray(x, np.float32)
    Wo32 = np.asarray(Wo, np.float32)
    bo_eff = np.asarray(bo, np.float32) + np.asarray(bv, np.float32) @ Wo32

    def q8(a, s):  # scale by 2^s then quantize e4m3
        return np.ascontiguousarray(
            (np.asarray(a, np.float32) * 2.0 ** s).astype(f8)
        )

    def col(b_, n, s=0):  # [n*128] -> [128, n] column layout
        return np.ascontiguousarray(
            (np.asarray(b_, np.float32) * 2.0 ** s).reshape(n, P).T
        )

    def row(b_, s=0):
        return np.ascontiguousarray(
            (np.asarray(b_, np.float32) * 2.0 ** s).reshape(1, -1)
        )

    shared = {
        "wq": q8(Wq, SW),
        "wk": q8(Wk, SW),
        "wv": q8(Wv, SW),
        "wo": q8(Wo32, SW),
        "w1": q8(W1, SW1),
        "w2": q8(W2, SW2),
        "bqc": col(bq, NDC, SQ),
        "bkc": col(bk, NDC, SQ),
        "b1c": col(b1, NFC, SU),
        "b2r": row(b2, SLN),
        "g1r": row(g1, SLN),
        "be1r": row(be1, SLN),
        "g2r": row(g2),
        "be2r": row(be2),
    }

    in_maps = []
    for c in range(8):
        b_, hf = c // 2, c % 2
        xb = x[b_]  # [2048, 1024]
        loc = xb[hf * S_LOC : (hf + 1) * S_LOC, :]
        rem = xb[(1 - hf) * S_LOC : (2 - hf) * S_LOC, :]
        m = dict(shared)
        # token axis rolled: local tokens first (keys are permutation-inv.)
        m["xt"] = np.ascontiguousarray(
            (np.concatenate([loc, rem], axis=0).T * 2.0 ** SX).astype(f8)
        )
        m["xloc"] = np.ascontiguousarray(
            (loc + bo_eff[None, :]) * 2.0 ** SRES
        )
        in_maps.append(m)
    return in_maps


_NC = {}
LAST_RESULTS = None  # BassKernelResults of the most recent kernel() call


def detect_ln_trivial(g1, be1, g2, be2, b2, **_):
    return bool(
        np.all(np.asarray(g1) == 1.0) and np.all(np.asarray(be1) == 0.0)
        and np.all(np.asarray(g2) == 1.0) and np.all(np.asarray(be2) == 0.0)
        and np.all(np.asarray(b2) == 0.0)
    )


def kernel(x, mask, Wq, bq, Wk, bk, Wv, bv, Wo, bo, W1, b1, W2, b2, g1, be1, g2, be2):
    triv = detect_ln_trivial(g1=g1, be1=be1, g2=g2, be2=be2, b2=b2)
    if triv not in _NC:
        _NC[triv] = build_program(ln_trivial=triv)
    nc = _NC[triv]

    in_maps = prepare_in_maps(x, mask, Wq, bq, Wk, bk, Wv, bv, Wo, bo,
                              W1, b1, W2, b2, g1, be1, g2, be2)

    res = run_bass_kernel_spmd(nc, in_maps, list(range(8)))
    global LAST_RESULTS
    LAST_RESULTS = res

    out = np.empty((4, S_FULL, D), np.float32)
    for c in range(8):
        b_, hf = c // 2, c % 2
        out[b_, hf * S_LOC : (hf + 1) * S_LOC, :] = res.results[c]["out"]
    return out
